# revision 3
# baseline (speedup 1.0000x reference)
"""CAM (channel attention) kernel for Trainium2, SPMD over 8 NeuronCores.

Computation per batch b (reference semantics):
    v      = x[b].reshape(C, N)                      # C=512, N=4096
    energy = v @ v.T                                 # [C, C] Gram over channels
    att    = softmax(max_j(energy) - energy, axis=-1)
           = exp(min_j(energy) - energy) / sum_j(...)   # algebraically identical
    out    = gamma * (att @ v) + x[b]

Distribution: pure data parallel over batch. B=16 -> 2 batches per core.

Per-core kernel design (per batch):
  - load v natural [C,N] as one SBUF tile [128, 4, 4096] (partition=channel%128)
  - PE-transposes build u_k = v[:, 128k:128(k+1)]^T tiles [128n, 512c] on the fly
  - energy m-tiles accumulate in 4 PSUM banks: e[m] += u_k[:, m-blk].T @ u_k
  - row-softmax: DVE row-min, ACT exp(bias=min, scale=-1) with fused row-sum,
    DVE reciprocal; gr = gamma / sum
  - att^T via 16 PE-transposes; out m-tiles: po = att^T[tj,ti-blk].T @ v[tj]
  - evacuation fuses scale+residual: final = (po * gr_i) + x chunk  (one DVE op)
All matmuls/transposes run in float32r (full-rate fp32 mode of the PE).
"""

import numpy as np

import concourse.bass as bass
import concourse.bacc as bacc
import concourse.tile as tile
from concourse import mybir
from concourse.bass_utils import run_bass_kernel_spmd
from concourse.masks import make_identity

F32 = mybir.dt.float32
F32R = mybir.dt.float32r

B, C, H, W = 16, 512, 64, 64
N = H * W                  # 4096
NCORES = 8
BPC = B // NCORES          # batches per core = 2
CT = C // 128              # 4 channel tiles
KT = N // 128              # 32 contraction tiles for the Gram matrix
FT = N // 512              # 8 free-dim chunks for the out matmul
LOAD_CHUNKS = 4            # split the per-batch v load into this many DMAs
TDEPTH = 2                 # transpose software-pipeline depth (k-tiles ahead)


def build():
    nc = bacc.Bacc(
        "TRN2",
        target_bir_lowering=False,
        debug=False,
        num_devices=NCORES,
    )
    x_d = nc.dram_tensor("x", [BPC, C, N], F32R, kind="ExternalInput")
    g_d = nc.dram_tensor("gamma", [1], F32, kind="ExternalInput")
    o_d = nc.dram_tensor("out", [BPC, C, N], F32, kind="ExternalOutput")
    x_ap, g_ap, o_ap = x_d.ap(), g_d.ap(), o_d.ap()

    with tile.TileContext(nc) as tc:
        with (
            tc.tile_pool(name="const", bufs=1) as const_pool,
            tc.tile_pool(name="vb", bufs=2) as v_pool,
            tc.tile_pool(name="u", bufs=TDEPTH + 2) as u_pool,
            tc.tile_pool(name="att", bufs=2) as att_pool,
            tc.tile_pool(name="attT", bufs=2) as attT_pool,
            tc.tile_pool(name="stage", bufs=4) as stage_pool,
            tc.tile_pool(name="stats", bufs=4) as stats_pool,
            tc.tile_pool(name="gr", bufs=2) as gr_pool,
            tc.tile_pool(name="epsum", bufs=1, space="PSUM") as e_pool,
            tc.tile_pool(name="tpsum", bufs=2, space="PSUM") as t_pool,
            tc.tile_pool(name="opsum", bufs=2, space="PSUM") as o_pool,
        ):
            ident = const_pool.tile([128, 128], F32)
            make_identity(nc, ident)
            identr = const_pool.tile([128, 128], F32R, name="identr")
            nc.scalar.copy(identr, ident)

            gam = const_pool.tile([128, 1], F32)
            nc.sync.dma_start(out=gam, in_=g_ap.to_broadcast((128, 1)))

            # per-batch state carried from phase 1 to phase 2
            state = {}

            def phase1(b):
                vb = v_pool.tile([128, CT, N], F32R, tag="vb")
                nsz = N // LOAD_CHUNKS
                xb = x_ap[b].rearrange("(c p) n -> p c n", p=128)
                for lc in range(LOAD_CHUNKS):
                    nsl = bass.ds(lc * nsz, nsz)
                    nc.sync.dma_start(out=vb[:, :, nsl], in_=xb[:, :, nsl])

                e = [
                    e_pool.tile([128, C], F32, tag=f"e{m}", name=f"e{m}")
                    for m in range(CT)
                ]

                def energy_mms(k, u):
                    for m in range(CT):
                        nc.tensor.matmul(
                            e[m],
                            u[:, bass.ts(m, 128)],
                            u,
                            start=(k == 0),
                            stop=(k == KT - 1),
                        )

                pending = []
                for k in range(KT):
                    up = t_pool.tile([128, C], F32, tag="upsum", name="upsum")
                    upr = up.bitcast(F32R)
                    for ci in range(CT):
                        nc.tensor.transpose(
                            upr[:, bass.ts(ci, 128)],
                            vb[:, ci, bass.ts(k, 128)],
                            identr,
                        )
                    u = u_pool.tile([128, C], F32R, tag="u", name="u")
                    nc.scalar.copy(u, up)
                    pending.append((k, u))
                    if len(pending) > TDEPTH:
                        energy_mms(*pending.pop(0))
                while pending:
                    energy_mms(*pending.pop(0))

                # row softmax (reversed-max form): att = exp(min - e) / sum
                att = []
                gr = []
                for m in range(CT):
                    mn = stats_pool.tile([128, 1], F32, tag="mn", name="mn")
                    nc.vector.tensor_reduce(
                        mn, e[m], axis=mybir.AxisListType.X, op=mybir.AluOpType.min
                    )
                    a = att_pool.tile([128, C], F32R, tag=f"att{m}", name=f"att{m}")
                    s = stats_pool.tile([128, 1], F32, tag="s", name="s")
                    nc.scalar.activation(
                        a,
                        e[m],
                        mybir.ActivationFunctionType.Exp,
                        bias=mn,
                        scale=-1.0,
                        accum_out=s,
                    )
                    r = stats_pool.tile([128, 1], F32, tag="r", name="r")
                    nc.vector.reciprocal(r, s)
                    g = gr_pool.tile([128, 1], F32, tag=f"gr{m}", name=f"gr{m}")
                    nc.vector.tensor_scalar_mul(g, r, gam[:, 0:1])
                    att.append(a)
                    gr.append(g)

                state[b] = (vb, att, gr)

            def phase2(b):
                vb, att, gr = state.pop(b)

                # att^T tiles: attT[tj][:, ti-blk] = att[ti][:, tj-blk]^T
                attT = []
                for tj in range(CT):
                    ap_ps = t_pool.tile([128, C], F32, tag="upsum", name="atpsum")
                    apr = ap_ps.bitcast(F32R)
                    for ti in range(CT):
                        nc.tensor.transpose(
                            apr[:, bass.ts(ti, 128)],
                            att[ti][:, bass.ts(tj, 128)],
                            identr,
                        )
                    at = attT_pool.tile([128, C], F32R, tag=f"attT{tj}", name=f"attT{tj}")
                    nc.scalar.copy(at, ap_ps)
                    attT.append(at)

                for ti in range(CT):
                    for f in range(FT):
                        fsl = bass.ts(f, 512)
                        po = o_pool.tile([128, 512], F32, tag="opsum", name="opsum")
                        for tj in range(CT):
                            nc.tensor.matmul(
                                po,
                                attT[tj][:, bass.ts(ti, 128)],
                                vb[:, tj, fsl],
                                start=(tj == 0),
                                stop=(tj == CT - 1),
                            )
                        stg = stage_pool.tile([128, 512], F32, tag="stage", name="stage")
                        # final = (po * (gamma/sum_i)) + x   in one DVE op
                        nc.vector.scalar_tensor_tensor(
                            stg,
                            po,
                            gr[ti][:, 0:1],
                            vb[:, ti, fsl].bitcast(F32),
                            op0=mybir.AluOpType.mult,
                            op1=mybir.AluOpType.add,
                        )
                        nc.scalar.dma_start(
                            out=o_ap[b, bass.ts(ti, 128), fsl], in_=stg
                        )

            # interleave: b1's transposes fill the PE gap left by b0's softmax
            for b in range(BPC):
                phase1(b)
                if b > 0:
                    phase2(b - 1)
            phase2(BPC - 1)

    nc.compile()
    if not nc.is_finalized():
        nc.finalize()
    return nc


_NC = None


def _get_nc():
    global _NC
    if _NC is None:
        _NC = build()
    return _NC


def _run(x, gamma, **kw):
    nc = _get_nc()
    x = np.ascontiguousarray(np.asarray(x, dtype=np.float32).reshape(B, C, N))
    g = np.asarray(gamma, dtype=np.float32).reshape(1)
    in_maps = [
        {"x": x[c * BPC : (c + 1) * BPC], "gamma": g} for c in range(NCORES)
    ]
    res = run_bass_kernel_spmd(nc, in_maps, list(range(NCORES)), **kw)
    out = np.concatenate([r["out"] for r in res.results], axis=0)
    return out.reshape(B, C, H, W), res


def kernel(x, gamma):
    out, _ = _run(x, gamma)
    return out
